# revision 1
# baseline (speedup 1.0000x reference)
"""Trainium2 Bass kernel for PetraRQ self-attention (linformer-style projected KV).

Math (per batch b):
    q  = x @ Wq;  keys = x @ Wk;  values = x @ Wv
    keys_p   = proj_k.T @ keys      (= (proj_k.T @ x) @ Wk, associativity trick)
    values_p = proj_v.T @ values    (= (proj_v.T @ x) @ Wv)
    per head: dots = q_h @ keys_p_h.T / sqrt(DH); attn = softmax(dots)
    out_h = attn @ values_p_h;  out = concat(out_h) @ Wo + bo

Sharding: data-parallel over batch, one batch element per NeuronCore (8 cores).

On-chip layout strategy (feature-major / transposed activations):
    xT  [D, N]   (host-transposed)    qT = Wq.T@... computed as [D, N]
    xp_kT [D, K] = (proj_k.T @ x).T   via x-natural stationary matmuls
    keys_pT [D, K], values_p [K, D]
    dotsT [K, N] per head -> exp (no max subtraction; |dots| <= ~7)
    U^T [DH, N] per head via lhsT=[v_h|1] -> row 64 = softmax denominator Z
    final = (U^T/Z stacked).T @ Wo + bo   computed natively as [N, D]
All matmuls in bf16 with fp32 PSUM accumulation (validated: rel absmax err ~0.8%).
"""

import sys

for _p in ("/opt/trn_rl_repo",):
    if _p not in sys.path:
        sys.path.insert(0, _p)

from contextlib import ExitStack

import ml_dtypes
import numpy as np

B, N, D = 8, 4096, 1024
H, DH, K = 16, 64, 256
P = 128
NB = 512  # n-block width for the fused q/attention phase
NCORES = 8


DEBUG_TAPS = False

# tunables (cost-model A/B)
P1X_BUFS = 6
P3XT_BUFS = 3
P3Q_BUFS = 2
P4E_BUFS = 4
P4Z_BUFS = 3
P3PS_BUFS = 2
P4PD_BUFS = 3
FUSE_P5 = True
PHASES = "12345"
P4PU_BUFS = 2
P5SB_BUFS = 2
P5PS_BUFS = 1


def build_body(ctx, tc, aps, n):
    import concourse.bass as bass  # noqa: F401
    from concourse import mybir
    from concourse.alu_op_type import AluOpType

    nc = tc.nc
    bf = mybir.dt.bfloat16
    f32 = mybir.dt.float32
    EC = D // P  # 8  e/d chunks
    KT = K // P  # 2  k tiles
    NCH = n // P  # x chunks (n on partitions)
    NBLK = n // NB  # n blocks
    HP = H // 2  # head pairs

    xt_d, xn_d, wq_d, wk_d, wv_d, wo_d, pkv_d, bo_d, y_d = aps[:9]

    def tap(name, ap):
        if DEBUG_TAPS:
            d = nc.dram_tensor(f"dbg_{name}", list(ap.shape), ap.dtype,
                               kind="ExternalOutput").ap()
            nc.sync.dma_start(d, ap)

    # ---------------- long-lived sbuf pools ----------------
    pool_r = ctx.enter_context(tc.tile_pool(name="resident", bufs=1))
    kpt_s = pool_r.tile([P, EC, K], bf, tag="kpt")  # keys_p^T  [e, k]
    vaug_s = pool_r.tile([P, KT, H, 2 * DH], bf, tag="vaug")  # [k, kt, h, dh|1x64]
    nc.any.memset(vaug_s[:, :, :, DH : 2 * DH], 1.0)

    # ---------------- phase 1: xp_kT / xp_vT  [d, k] ----------------
    # xp_kT[d, k] = sum_n x[n, d] pk[n, k]; lhsT = x natural chunk, rhs = pk chunk
    with tc.tile_pool(name="p1sb", bufs=1) as p1sb:
      if "1" in PHASES:
        pkv_s = p1sb.tile([P, NCH, 2 * K], bf, tag="pkv")
        xpkv_s = p1sb.tile([P, EC, 2 * K], bf, tag="xpkv")
        for q in range(4):
            nc.sync.dma_start(pkv_s[:, q * (NCH // 4) : (q + 1) * (NCH // 4), :],
                              pkv_d[:, q * (NCH // 4) : (q + 1) * (NCH // 4), :])
        XG = 4  # x chunks per DMA
        with tc.tile_pool(name="p1x", bufs=P1X_BUFS) as p1x, \
             tc.tile_pool(name="p1ps", bufs=1, space="PSUM") as p1ps:
            ps = {dc: p1ps.tile([P, 2 * K], f32, tag=f"ps{dc}",
                                name=f"ps_{dc}") for dc in range(EC)}
            for ng in range(NCH // XG):
                xg = p1x.tile([P, XG, D], bf, tag="xg", name=f"xg_{ng}")
                nc.sync.dma_start(xg[:], xn_d[:, ng * XG : (ng + 1) * XG, :])
                for j in range(XG):
                    nch = ng * XG + j
                    for dc in range(EC):
                        nc.tensor.matmul(ps[dc][:],
                                         xg[:, j, dc * P : (dc + 1) * P],
                                         pkv_s[:, nch, :],
                                         start=(nch == 0), stop=(nch == NCH - 1))
            for dc in range(EC):
                nc.any.tensor_copy(xpkv_s[:, dc, :], ps[dc][:])
        xpk_s = xpkv_s[:, :, 0:K]
        xpv_s = xpkv_s[:, :, K : 2 * K]
        tap("xpk", xpk_s[:])
        tap("xpv", xpv_s[:])

        # ---------------- phase 2: keys_pT [e, k], values_p [k, e] ----------
        with tc.tile_pool(name="p2sb", bufs=1) as p2sb, \
             tc.tile_pool(name="p2ps", bufs=2, space="PSUM") as p2ps:
            wk_s = p2sb.tile([P, EC, D], bf, tag="wk")
            wv_s = p2sb.tile([P, EC, D], bf, tag="wv")
            nc.sync.dma_start(wk_s[:], wk_d)
            nc.sync.dma_start(wv_s[:], wv_d)
            for ec in range(EC):
                pko = p2ps.tile([P, K], f32, tag="pko")
                for dc in range(EC):
                    nc.tensor.matmul(pko[:], wk_s[:, dc, ec * P : (ec + 1) * P],
                                     xpkv_s[:, dc, 0:K],
                                     start=(dc == 0), stop=(dc == EC - 1))
                nc.any.tensor_copy(kpt_s[:, ec, :], pko[:])
            for kt in range(KT):
                for eb in range(D // 512):
                    pvo = p2ps.tile([P, 512], f32, tag="pvo")
                    for dc in range(EC):
                        nc.tensor.matmul(
                            pvo[:], xpkv_s[:, dc, K + kt * P : K + (kt + 1) * P],
                            wv_s[:, dc, eb * 512 : (eb + 1) * 512],
                            start=(dc == 0), stop=(dc == EC - 1))
                    nc.any.tensor_copy(
                        vaug_s[:, kt, eb * 8 : (eb + 1) * 8, 0:DH],
                        pvo[:].rearrange("p (h dh) -> p h dh", dh=DH))
            tap("kpt", kpt_s[:])
            tap("vaug", vaug_s[:])

    # ---------------- fused phase 3+4 per n-block ----------------
    # qT block [e, NB], then per head: dotsT -> exp -> U^T(+Zx64) -> normalize
    pool_u = ctx.enter_context(tc.tile_pool(name="poolu", bufs=1))
    ut_s = pool_u.tile([P, EC, n], bf, tag="ut")  # normalized U^T [e, n]
    wo_s = pool_u.tile([P, EC, D], bf, tag="wo")
    bo_s = pool_u.tile([P, D], f32, tag="bo")
    nc.sync.dma_start(wo_s[:], wo_d)
    nc.sync.dma_start(bo_s[:], bo_d)
    with tc.tile_pool(name="p3sb", bufs=1) as p3sb, \
         tc.tile_pool(name="p3xt", bufs=P3XT_BUFS) as p3xt, \
         tc.tile_pool(name="p3q", bufs=P3Q_BUFS) as p3q, \
         tc.tile_pool(name="p4e", bufs=P4E_BUFS) as p4e, \
         tc.tile_pool(name="p4z", bufs=P4Z_BUFS) as p4z, \
         tc.tile_pool(name="p4stg", bufs=2) as p4stg, \
         tc.tile_pool(name="p5sb", bufs=P5SB_BUFS) as p5sb, \
         tc.tile_pool(name="p3ps", bufs=P3PS_BUFS, space="PSUM") as p3ps, \
         tc.tile_pool(name="p4pd", bufs=P4PD_BUFS, space="PSUM") as p4pd, \
         tc.tile_pool(name="p4pu", bufs=P4PU_BUFS, space="PSUM") as p4pu, \
         tc.tile_pool(name="p5ps", bufs=P5PS_BUFS, space="PSUM") as p5ps:
        wq_s = p3sb.tile([P, EC, D], bf, tag="wq")
        nc.sync.dma_start(wq_s[:], wq_d)

        def p5_block(nb):
            # y tiles for the 128-row strips covered by finished block nb
            for nt in range(nb * (NB // P), (nb + 1) * (NB // P)):
                o_s = p5sb.tile([P, D], f32, tag="os", name=f"os_{nt}")
                for db in range(D // 512):
                    pf = p5ps.tile([P, 512], f32, tag="pf", name=f"pf_{nt}_{db}")
                    for ec in range(EC):
                        nc.tensor.matmul(
                            pf[:], ut_s[:, ec, nt * P : (nt + 1) * P],
                            wo_s[:, ec, db * 512 : (db + 1) * 512],
                            start=(ec == 0), stop=(ec == EC - 1))
                    nc.vector.tensor_add(o_s[:, db * 512 : (db + 1) * 512], pf[:],
                                         bo_s[:, db * 512 : (db + 1) * 512])
                nc.gpsimd.dma_start(y_d[nt * P : (nt + 1) * P, :], o_s[:])
        for nb in range(NBLK):
            if "3" not in PHASES:
                break
            nbs = slice(nb * NB, (nb + 1) * NB)
            xtb = p3xt.tile([P, EC, NB], bf, tag="xtb")
            nc.sync.dma_start(xtb[:], xt_d[:, :, nbs])
            qtb = p3q.tile([P, EC, NB], bf, tag="qtb")
            stgb = p4stg.tile([64, HP, NB], bf, tag="stgb")
            for ec in range(EC):
                psq = p3ps.tile([P, NB], f32, tag="psq")
                for dc in range(EC):
                    nc.tensor.matmul(psq[:], wq_s[:, dc, ec * P : (ec + 1) * P],
                                     xtb[:, dc, :],
                                     start=(dc == 0), stop=(dc == EC - 1))
                nc.any.tensor_copy(qtb[:, ec, :], psq[:])
            if nb == 0:
                tap("qtb0", qtb[:])
            for hp in range(HP):
                if "4" not in PHASES:
                    break
                ets = []
                for hi in range(2):
                    et = p4e.tile([P, KT, NB], bf, tag=f"et{hi}",
                                  name=f"et_{hi}")
                    ets.append(et)
                for kt in range(KT):
                    for hi in range(2):  # two heads, row-groups 0-63 / 64-127
                        base = 64 * hi
                        pd = p4pd.tile([P, NB], f32, tag="pd",
                                       name=f"pd_{hi}_{kt}")
                        nc.tensor.matmul(
                            pd[:],
                            kpt_s[base : base + 64, hp, kt * P : (kt + 1) * P],
                            qtb[base : base + 64, hp, :],
                            start=True, stop=True)
                        nc.scalar.activation(ets[hi][:, kt, :], pd[:],
                                             mybir.ActivationFunctionType.Exp)
                for hi in range(2):
                    h = 2 * hp + hi
                    base = 64 * hi
                    et = ets[hi]
                    if nb == 0 and hp == 0:
                        tap(f"et{hi}", et[:])
                    pu = p4pu.tile([2 * DH, NB], f32, tag="pu")
                    for kt in range(KT):
                        nc.tensor.matmul(pu[:], vaug_s[:, kt, h, :], et[:, kt, :],
                                         start=(kt == 0), stop=(kt == KT - 1))
                    # rows 64..127 of pu are all Z (64 replicated ones cols)
                    zinv = p4z.tile([64, NB], f32, tag="zinv")
                    nc.vector.reciprocal(zinv[:], pu[DH : 2 * DH, :])
                    if nb == 0 and hp == 0:
                        tap(f"zb{hi}", zinv[:])
                    if hi == 0:
                        nc.vector.tensor_tensor(ut_s[0:64, hp, nbs], pu[0:DH, :],
                                                zinv[:], AluOpType.mult)
                    else:
                        nc.vector.tensor_tensor(stgb[:, hp, :], pu[0:DH, :],
                                                zinv[:], AluOpType.mult)
            if "4" in PHASES:
                nc.gpsimd.dma_start(ut_s[64:128, :, nbs], stgb[:])
            if FUSE_P5 and "5" in PHASES:
                p5_block(nb)
        if not FUSE_P5 and "5" in PHASES:
            for nb in range(NBLK):
                p5_block(nb)
        tap("ut", ut_s[:])


def build_kernel(n=N, loops=1):
    import concourse.bacc as bacc
    import concourse.tile as tile
    from concourse import mybir

    bf = mybir.dt.bfloat16
    f32 = mybir.dt.float32
    nc = bacc.Bacc("TRN2", target_bir_lowering=False, debug=False)
    aps = [
        nc.dram_tensor("xt", [P, D // P, n], bf, kind="ExternalInput").ap(),
        nc.dram_tensor("xn", [P, n // P, D], bf, kind="ExternalInput").ap(),
        nc.dram_tensor("wq", [P, D // P, D], bf, kind="ExternalInput").ap(),
        nc.dram_tensor("wk", [P, D // P, D], bf, kind="ExternalInput").ap(),
        nc.dram_tensor("wv", [P, D // P, D], bf, kind="ExternalInput").ap(),
        nc.dram_tensor("wo", [P, D // P, D], bf, kind="ExternalInput").ap(),
        nc.dram_tensor("pkv", [P, n // P, 2 * K], bf, kind="ExternalInput").ap(),
        nc.dram_tensor("bo", [P, D], f32, kind="ExternalInput").ap(),
        nc.dram_tensor("y", [n, D], f32, kind="ExternalOutput").ap(),
    ]
    with tile.TileContext(nc) as tc:
        for _ in range(loops):
            with ExitStack() as ctx:
                build_body(ctx, tc, aps, n)
    nc.compile()
    return nc


def make_in_maps(x, Wq, Wk, Wv, proj_k, proj_v, Wo, bo, n=N, b=B):
    bfn = ml_dtypes.bfloat16

    def dmaj(w):  # [D, E] -> [P, D//P, E]
        return np.ascontiguousarray(
            w.reshape(D // P, P, -1).transpose(1, 0, 2)).astype(bfn)

    def nmaj(a, n_):  # [n, C] -> [P, n//P, C]
        return np.ascontiguousarray(
            a.reshape(n_ // P, P, -1).transpose(1, 0, 2)).astype(bfn)

    shared = {
        "wq": dmaj(np.asarray(Wq) * (DH ** -0.5)),
        "wk": dmaj(np.asarray(Wk)),
        "wv": dmaj(np.asarray(Wv)),
        "wo": dmaj(np.asarray(Wo)),
        "pkv": nmaj(np.concatenate([np.asarray(proj_k), np.asarray(proj_v)], axis=1), n),
        "bo": np.ascontiguousarray(
            np.broadcast_to(np.asarray(bo, np.float32), (P, D))),
    }
    in_maps = []
    for bi in range(b):
        xb = np.asarray(x[bi], np.float32)
        in_maps.append({
            **shared,
            "xt": dmaj(np.ascontiguousarray(xb.T)),
            "xn": nmaj(xb, n),
        })
    return in_maps


_NC_CACHE = {}


def _get_nc(n=N):
    if n not in _NC_CACHE:
        _NC_CACHE[n] = build_kernel(n)
    return _NC_CACHE[n]


def kernel(x, Wq, Wk, Wv, proj_k, proj_v, Wo, bo):
    from concourse.bass_utils import run_bass_kernel_spmd

    nc = _get_nc(N)
    in_maps = make_in_maps(x, Wq, Wk, Wv, proj_k, proj_v, Wo, bo)
    res = run_bass_kernel_spmd(nc, in_maps, list(range(NCORES)))
    return np.stack([res.results[i]["y"] for i in range(NCORES)])


if __name__ == "__main__":
    rng = np.random.default_rng(0)
    x = rng.standard_normal((B, N, D), dtype=np.float32)
    Wq = rng.standard_normal((D, D), dtype=np.float32) * 0.02
    Wk = rng.standard_normal((D, D), dtype=np.float32) * 0.02
    Wv = rng.standard_normal((D, D), dtype=np.float32) * 0.02
    pk = rng.standard_normal((N, K), dtype=np.float32) * 0.05
    pv = rng.standard_normal((N, K), dtype=np.float32) * 0.05
    Wo = rng.standard_normal((D, D), dtype=np.float32) * 0.02
    bo = rng.standard_normal((D,), dtype=np.float32)
    out = kernel(x, Wq, Wk, Wv, pk, pv, Wo, bo)
    print(out.shape, out.dtype)

